# revision 1
# baseline (speedup 1.0000x reference)
"""Trainium2 Bass kernel for nn_ExpertGroup (moe_routing).

Contract: kernel(**inputs) takes FULL unsharded numpy inputs and returns the
FULL [2, 2048, 1024] fp32 output. Internally shards B*S=4096 tokens across
8 NeuronCores (512 tokens/core; cores 0-3 own batch 0, cores 4-7 batch 1),
replicates the small weights, and exchanges the per-batch adapter tensors
(adapt_in / adapt_out, [S,128] each) with two intra-group AllGathers.

All matmuls run in bf16 with fp32 PSUM accumulation. Activations are laid out
feature-major ([feature, token]) so every matmul contracts over partitions.

Host-side algebraic folds (exact, input-dependent, valid for any inputs):
  - up/gate weights concatenated into one [D, 2H] lhsT
  - shared_out + out = hidden @ down_w.T + adapt @ (0.1*down_w@adapt_proj_w).T
                      + mixed @ (output_proj_w@expert_proj_w).T
    -> single PSUM accumulation over 16+1+1 K-chunks of 128
  - sum_e ew[n,e]*adapter_b[e,:]  (LN bias term) = expert_weights @ adapter_b
"""

import sys

sys.path.insert(0, "/opt/trn_rl_repo")

import ml_dtypes
import numpy as np

import concourse.bass as bass
import concourse.mybir as mybir
import concourse.tile as tile
from concourse import bacc
from concourse.bass_utils import run_bass_kernel_spmd

BF16 = mybir.dt.bfloat16
F32 = mybir.dt.float32

B, S, D, E = 2, 2048, 1024, 8
H = 2 * D          # 2048
A = H // 16        # 128
N = B * S          # 4096
NCORES = 8
T = N // NCORES    # 512 tokens per core
GROUP = 4          # cores per batch
SC = T // 128      # 4 s-chunks per core
DC = D // 128      # 8 d-chunks (output features)
HC = H // 128      # 16 h-chunks
KD = D // 128      # 8 k-chunks over D
TC_FULL = S // 128  # 16 token-chunks per batch
EPS = 1e-5

_CACHE = {}


def _build():
    nc = bacc.Bacc(None, num_devices=NCORES)

    # ---- kernel I/O (per-core; weights pre-packed to SBUF layout on host) ----
    xT_d = nc.dram_tensor("xT", [128, KD, T], BF16, kind="ExternalInput")
    ug_d = nc.dram_tensor("ug_wT", [128, KD, 2 * H], BF16, kind="ExternalInput")
    pre_d = nc.dram_tensor("pre_wT", [128, KD, A], BF16, kind="ExternalInput")
    post_d = nc.dram_tensor("post_wT", [128, A, HC], BF16, kind="ExternalInput")
    adw_d = nc.dram_tensor("adapter_wT", [A, E * A], BF16, kind="ExternalInput")
    wfin_d = nc.dram_tensor("wfin", [128, HC + 2, D], BF16, kind="ExternalInput")
    ew_d = nc.dram_tensor("ew", [128, SC, E], F32, kind="ExternalInput")
    angb_d = nc.dram_tensor("angb", [2, A], F32, kind="ExternalInput")
    ancol_d = nc.dram_tensor("ancol", [128, 2], F32, kind="ExternalInput")
    ag_d = nc.dram_tensor("ag_row", [1, A * E], BF16, kind="ExternalInput")
    bmix_d = nc.dram_tensor("bias_mix", [128, SC, A], BF16, kind="ExternalInput")
    out_d = nc.dram_tensor("out", [D, T], F32, kind="ExternalOutput")

    # ---- collective bounce buffers (internal DRAM) ----
    ag0_in = nc.dram_tensor("ag0_in", [1, 8], BF16)
    ag0_out = nc.dram_tensor("ag0_out", [GROUP, 8], BF16)
    ag1_in = nc.dram_tensor("ag1_in", [T, A], BF16)
    ag1_out = nc.dram_tensor("ag1_out", [S, A], BF16)
    ag2_in = nc.dram_tensor("ag2_in", [A, T], BF16)
    ag2_out = nc.dram_tensor("ag2_out", [GROUP * A, T], BF16)
    RG = [[0, 1, 2, 3], [4, 5, 6, 7]]

    with tile.TileContext(nc) as tc:
        with (
            tc.tile_pool(name="consts", bufs=1) as consts,
            tc.tile_pool(name="wpool", bufs=1) as wpool,
            tc.tile_pool(name="acts", bufs=1) as acts,
            tc.tile_pool(name="work", bufs=4) as work,
            tc.tile_pool(name="work2", bufs=2) as work2,
            tc.tile_pool(name="wtp", bufs=3) as wtp,
            tc.tile_pool(name="workbig", bufs=1) as workbig,
            tc.tile_pool(name="aoln", bufs=1) as aoln,
            tc.tile_pool(name="evac", bufs=2) as evac,
            tc.tile_pool(name="ps_big", bufs=3, space="PSUM") as ps_big,
            tc.tile_pool(name="ps_po", bufs=1, space="PSUM") as ps_po,
            tc.tile_pool(name="ps_acc", bufs=1, space="PSUM") as ps_acc,
            tc.tile_pool(name="ps_out", bufs=2, space="PSUM") as ps_out,
            tc.tile_pool(name="ps_sm", bufs=1, space="PSUM") as ps_sm,
        ):
            # warm up the collective subsystem: the first collective pays the
            # all-core entry barrier (20-70us); absorb it during the DMA ramp
            warm = consts.tile([1, 8], BF16)
            nc.vector.memset(warm, 0.0)
            nc.gpsimd.dma_start(out=ag0_in[:], in_=warm)
            nc.gpsimd.collective_compute(
                "AllGather", mybir.AluOpType.bypass, replica_groups=RG,
                ins=[ag0_in[:]], outs=[ag0_out[:]],
            )

            # ---------- priority loads: the pre-matmul chain needs these ----
            xT = wpool.tile([128, KD, T], BF16)
            pre_w = wpool.tile([128, KD, A], BF16)
            nc.sync.dma_start(out=xT, in_=xT_d[:])
            nc.sync.dma_start(out=pre_w, in_=pre_d[:])

            # ---------- weight loads (split per chunk so consumers start early) --
            ug_w = wpool.tile([128, KD, 2 * H], BF16)
            for k in range(KD):
                nc.sync.dma_start(out=ug_w[:, k, :], in_=ug_d[:, k, :])
            post_w = wpool.tile([128, A, HC], BF16)
            nc.sync.dma_start(out=post_w, in_=post_d[:])
            adw = wpool.tile([128, E * A], BF16)
            nc.sync.dma_start(out=adw, in_=adw_d[:])
            wfin = wpool.tile([128, HC + 2, D], BF16)
            for k in range(HC + 2):
                nc.sync.dma_start(out=wfin[:, k, :], in_=wfin_d[:, k, :])

            # ---------- constants ----------
            eps_t = consts.tile([128, 1], F32)
            nc.vector.memset(eps_t, EPS)
            gB = consts.tile([128, A], F32)   # an_g broadcast across partitions
            bB = consts.tile([128, A], F32)   # an_b broadcast
            nc.sync.dma_start(
                out=gB,
                in_=bass.AP(tensor=angb_d, offset=0, ap=[[0, 128], [1, A]]),
            )
            nc.sync.dma_start(
                out=bB,
                in_=bass.AP(tensor=angb_d, offset=A, ap=[[0, 128], [1, A]]),
            )
            agB = consts.tile([128, E, A], BF16)  # adapter_g (e-major) bcast
            nc.sync.dma_start(
                out=agB,
                in_=bass.AP(tensor=ag_d, offset=0, ap=[[0, 128], [A, E], [1, A]]),
            )
            ancol = consts.tile([128, 2], F32)
            nc.sync.dma_start(out=ancol, in_=ancol_d[:])
            ones_col = consts.tile([128, 1], BF16)
            nc.vector.memset(ones_col, 1.0)
            ones_row = consts.tile([1, 128], BF16)
            nc.vector.memset(ones_row, 1.0)
            ew_sb = consts.tile([128, SC, E], F32)
            nc.sync.dma_start(out=ew_sb, in_=ew_d[:])
            bmix_sb = consts.tile([128, SC, A], BF16)
            nc.sync.dma_start(out=bmix_sb, in_=bmix_d[:])

            # persistent activations
            AI_tok = acts.tile([128, SC, A], BF16)    # adapt_in, token-major
            AIT = acts.tile([128, T], BF16)           # adapt_in, feature-major
            hid = acts.tile([128, HC, T], BF16)       # hidden, feature-major
            AOTfull = acts.tile([128, GROUP, T], BF16)   # gathered AO feat-major
            AOT = acts.tile([128, T], BF16)           # local AO, feature-major
            adaptT = acts.tile([128, T], BF16)        # adapt, feature-major
            mixedT = acts.tile([128, T], BF16)        # mixed, feature-major
            mix_tok = acts.tile([128, SC, A], BF16)   # mixed, token-major
            facc = acts.tile([128, DC, T], F32)       # down-part accumulator

            def layernorm_to(ps, dst):
                """LN over free dim (A=128) of psum tile [128, A]; write dst bf16."""
                st = work.tile([128, 6], F32, tag="lnst")
                nc.vector.bn_stats(out=st, in_=ps)
                mv = work.tile([128, 2], F32, tag="lnmv")
                nc.vector.bn_aggr(out=mv, in_=st)
                sd = work.tile([128, 1], F32, tag="lnsd")
                nc.scalar.activation(
                    out=sd, in_=mv[:, 1:2], func=mybir.ActivationFunctionType.Sqrt,
                    bias=eps_t, scale=1.0,
                )
                r = work.tile([128, 1], F32, tag="lnr")
                nc.vector.reciprocal(out=r, in_=sd)
                z = work.tile([128, A], F32, tag="lnz")
                nc.vector.tensor_scalar(
                    out=z, in0=ps, scalar1=mv[:, 0:1], scalar2=r,
                    op0=mybir.AluOpType.subtract, op1=mybir.AluOpType.mult,
                )
                zg = work.tile([128, A], F32, tag="lnzg")
                nc.vector.tensor_tensor(out=zg, in0=z, in1=gB, op=mybir.AluOpType.mult)
                nc.vector.tensor_tensor(out=dst, in0=zg, in1=bB, op=mybir.AluOpType.add)

            # ---------- adapt_in = LN(x @ pre_w.T), then AllGather #1 ----------
            for sc in range(SC):
                ps = ps_sm.tile([128, A], F32, tag="sm")
                for k in range(KD):
                    nc.tensor.matmul(
                        ps, xT[:, k, sc * 128:(sc + 1) * 128], pre_w[:, k, :],
                        start=(k == 0), stop=(k == KD - 1),
                    )
                layernorm_to(ps, AI_tok[:, sc, :])
            nc.gpsimd.dma_start(
                out=ag1_in[:].rearrange("(sc p) a -> p sc a", p=128), in_=AI_tok
            )
            nc.gpsimd.collective_compute(
                "AllGather", mybir.AluOpType.bypass, replica_groups=RG,
                ins=[ag1_in[:]], outs=[ag1_out[:]],
            )
            AIfull = acts.tile([128, TC_FULL, A], BF16)   # gathered AI token-major
            nc.gpsimd.dma_start(
                out=AIfull, in_=ag1_out[:].rearrange("(k p) a -> p k a", p=128)
            )

            # ---------- hidden = silu(x@gate.T) * (x@up.T), feature-major -------
            # post contraction (adapt_out pre-LN, feature-major) interleaved with
            # one-iteration delay so PE never waits on the DVE mul.
            po_ps = ps_po.tile([128, T], F32, tag="po")

            def post_step(k):
                nc.tensor.matmul(
                    po_ps, post_w[:, :, k], hid[:, k, :],
                    start=(k == 0), stop=(k == HC - 1),
                )

            for hc in range(HC):
                up_ps = ps_big.tile([128, T], F32, tag="mm")
                gt_ps = ps_big.tile([128, T], F32, tag="mm")
                for k in range(KD):
                    nc.tensor.matmul(
                        up_ps, ug_w[:, k, hc * 128:(hc + 1) * 128], xT[:, k, :],
                        start=(k == 0), stop=(k == KD - 1),
                    )
                for k in range(KD):
                    nc.tensor.matmul(
                        gt_ps, ug_w[:, k, H + hc * 128:H + (hc + 1) * 128],
                        xT[:, k, :], start=(k == 0), stop=(k == KD - 1),
                    )
                sg = work2.tile([128, T], BF16, tag="sg")
                nc.scalar.activation(
                    out=sg, in_=gt_ps, func=mybir.ActivationFunctionType.Silu
                )
                nc.vector.tensor_tensor(
                    out=hid[:, hc, :], in0=sg, in1=up_ps, op=mybir.AluOpType.mult
                )
                if hc > 0:
                    post_step(hc - 1)
            post_step(HC - 1)

            # ---------- adapt_out LN, feature-major (stats via PE ones-matmul) --
            AOf = acts.tile([128, T], BF16)
            nc.scalar.copy(out=AOf, in_=po_ps)
            sqf = aoln.tile([128, T], BF16)
            nc.vector.tensor_tensor(out=sqf, in0=AOf, in1=AOf, op=mybir.AluOpType.mult)
            s0 = ps_sm.tile([1, T], F32, tag="sm")
            nc.tensor.matmul(s0, ones_col, AOf, start=True, stop=True)
            s1 = ps_sm.tile([1, T], F32, tag="sm")
            nc.tensor.matmul(s1, ones_col, sqf, start=True, stop=True)
            mean_b = aoln.tile([1, T], BF16)
            nc.vector.tensor_scalar(
                out=mean_b, in0=s0, scalar1=1.0 / A, scalar2=None,
                op0=mybir.AluOpType.mult,
            )
            varf = aoln.tile([1, T], F32)
            nc.vector.tensor_scalar(
                out=varf, in0=s1, scalar1=1.0 / A, scalar2=None,
                op0=mybir.AluOpType.mult,
            )
            m2 = aoln.tile([1, T], F32, tag="fb")
            nc.vector.tensor_tensor(out=m2, in0=mean_b, in1=mean_b,
                                    op=mybir.AluOpType.mult)
            nc.vector.tensor_tensor(out=varf, in0=varf, in1=m2,
                                    op=mybir.AluOpType.subtract)
            sdf = aoln.tile([1, T], F32, tag="fa")
            nc.scalar.activation(
                out=sdf, in_=varf, func=mybir.ActivationFunctionType.Sqrt,
                bias=eps_t[0:1], scale=1.0,
            )
            rstd_f = aoln.tile([1, T], F32, tag="fb")
            nc.vector.reciprocal(out=rstd_f, in_=sdf)
            rstd_b = aoln.tile([1, T], BF16)
            nc.vector.tensor_copy(out=rstd_b, in_=rstd_f)
            meanB = ps_sm.tile([128, T], F32, tag="sm")
            nc.tensor.matmul(meanB, ones_row, mean_b, start=True, stop=True)
            rstdB = ps_sm.tile([128, T], F32, tag="sm")
            nc.tensor.matmul(rstdB, ones_row, rstd_b, start=True, stop=True)
            z1 = aoln.tile([128, T], BF16)
            nc.vector.tensor_tensor(out=z1, in0=AOf, in1=meanB,
                                    op=mybir.AluOpType.subtract)
            z2 = aoln.tile([128, T], BF16)
            nc.vector.tensor_tensor(out=z2, in0=z1, in1=rstdB,
                                    op=mybir.AluOpType.mult)
            nc.vector.tensor_scalar(
                out=AOT, in0=z2, scalar1=ancol[:, 0:1], scalar2=ancol[:, 1:2],
                op0=mybir.AluOpType.mult, op1=mybir.AluOpType.add,
            )
            nc.gpsimd.dma_start(out=ag2_in[:], in_=AOT)
            nc.gpsimd.collective_compute(
                "AllGather", mybir.AluOpType.bypass, replica_groups=RG,
                ins=[ag2_in[:]], outs=[ag2_out[:]],
            )
            for c in range(GROUP):
                nc.gpsimd.dma_start(
                    out=AOTfull[:, c, :], in_=ag2_out[c * A:(c + 1) * A, :]
                )
            AOTf = AOTfull.rearrange("a c t -> a (c t)")

            # transpose AI -> feature-major AIT (xbar DMA, off the weight-load
            # window; needed from the expert path onward)
            for sc in range(SC):
                nc.sync.dma_start_transpose(
                    out=AIT[:, sc * 128:(sc + 1) * 128], in_=AI_tok[:, sc, :]
                )

            # ---------- expert path (local tokens only; fills the AG2 window) ---
            for sc in range(SC):
                hp0 = ps_big.tile([128, 512], F32, tag="mm")
                hp1 = ps_big.tile([128, 512], F32, tag="mm")
                sl = AIT[:, sc * 128:(sc + 1) * 128]
                nc.tensor.matmul(hp0, sl, adw[:, 0:512], start=True, stop=True)
                nc.tensor.matmul(hp1, sl, adw[:, 512:1024], start=True, stop=True)
                hps = [hp0, hp0, hp0, hp0, hp1, hp1, hp1, hp1]
                st8 = work.tile([128, E, 6], F32, tag="st8")
                for e in range(E):
                    nc.vector.bn_stats(
                        out=st8[:, e, :], in_=hps[e][:, (e % 4) * A:(e % 4 + 1) * A]
                    )
                mv8 = work.tile([128, E, 2], F32, tag="mv8")
                for e in range(E):
                    nc.vector.bn_aggr(out=mv8[:, e, :], in_=st8[:, e, :])
                sd8 = work.tile([128, E], F32, tag="sd8")
                nc.scalar.activation(
                    out=sd8, in_=mv8[:, :, 1], func=mybir.ActivationFunctionType.Sqrt,
                    bias=eps_t, scale=1.0,
                )
                r8 = work.tile([128, E], F32, tag="r8")
                nc.vector.reciprocal(out=r8, in_=sd8)
                rw8 = work.tile([128, E], F32, tag="rw8")
                nc.vector.tensor_tensor(
                    out=rw8, in0=r8, in1=ew_sb[:, sc, :], op=mybir.AluOpType.mult
                )
                nmrw = work.tile([128, E], F32, tag="nmrw")
                nc.vector.tensor_tensor(
                    out=nmrw, in0=mv8[:, :, 0], in1=rw8, op=mybir.AluOpType.mult
                )
                nc.vector.tensor_scalar(
                    out=nmrw, in0=nmrw, scalar1=-1.0, scalar2=None,
                    op0=mybir.AluOpType.mult,
                )
                # z~_e = h_e * (r*ew)_e - m*(r*ew)_e, written e-outer [s, e, c]
                zt = workbig.tile([128, E, A], BF16, tag="zt")
                for e in range(E):
                    nc.scalar.activation(
                        out=zt[:, e, :], in_=hps[e][:, (e % 4) * A:(e % 4 + 1) * A],
                        func=mybir.ActivationFunctionType.Identity,
                        scale=rw8[:, e:e + 1], bias=nmrw[:, e:e + 1],
                    )
                zg = workbig.tile([128, E, A], BF16, tag="ztg")
                nc.vector.tensor_tensor(
                    out=zg, in0=zt, in1=agB, op=mybir.AluOpType.mult
                )
                t1 = workbig.tile([128, 4, A], BF16, tag="sum1")
                nc.vector.tensor_tensor(
                    out=t1, in0=zg[:, 0:4, :], in1=zg[:, 4:8, :],
                    op=mybir.AluOpType.add,
                )
                t2 = work.tile([128, 2, A], BF16, tag="sum2")
                nc.vector.tensor_tensor(
                    out=t2, in0=t1[:, 0:2, :], in1=t1[:, 2:4, :],
                    op=mybir.AluOpType.add,
                )
                mx = work.tile([128, A], BF16, tag="mx")
                nc.vector.tensor_tensor(
                    out=mx, in0=t2[:, 0, :], in1=t2[:, 1, :], op=mybir.AluOpType.add
                )
                nc.vector.tensor_tensor(
                    out=mix_tok[:, sc, :], in0=mx, in1=bmix_sb[:, sc, :],
                    op=mybir.AluOpType.add,
                )
            for sc in range(SC):
                nc.sync.dma_start_transpose(
                    out=mixedT[:, sc * 128:(sc + 1) * 128], in_=mix_tok[:, sc, :]
                )

            # ---------- final output down-part for ALL chunks (AG2 window) -----
            def final_down(dc):
                op = ps_out.tile([128, T], F32, tag="fout")
                for k in range(HC):
                    nc.tensor.matmul(
                        op, wfin[:, k, dc * 128:(dc + 1) * 128], hid[:, k, :],
                        start=(k == 0), stop=(k == HC - 1),
                    )
                nc.scalar.copy(out=facc[:, dc, :], in_=op)

            for dc in range(DC):
                final_down(dc)

            # ---------- w = silu(clip(AI_loc @ AO_full.T)); adapt = w.T chain ---
            ad_ps = ps_acc.tile([128, T], F32, tag="adps")
            wts_buf = {}

            def bmm1_step(j):
                w_ps = ps_big.tile([128, T], F32, tag="mm")
                nc.tensor.matmul(
                    w_ps, AOTf[:, j * 128:(j + 1) * 128], AIT, start=True, stop=True
                )
                wc = work2.tile([128, T], BF16, tag="wc")
                nc.vector.tensor_scalar(
                    out=wc, in0=w_ps, scalar1=-5.0, scalar2=5.0,
                    op0=mybir.AluOpType.max, op1=mybir.AluOpType.min,
                )
                wt = wtp.tile([128, T], BF16, tag="wts")
                nc.scalar.activation(
                    out=wt, in_=wc, func=mybir.ActivationFunctionType.Silu
                )
                wts_buf[j] = wt

            # depth-2 software pipeline: bmm2(j) trails bmm1(j+2) so the PE
            # never waits on the clip/silu stages
            for j in range(TC_FULL):
                bmm1_step(j)
                if j >= 2:
                    nc.tensor.matmul(
                        ad_ps, AIfull[:, j - 2, :], wts_buf.pop(j - 2),
                        start=(j - 2 == 0), stop=False,
                    )
            for j in (TC_FULL - 2, TC_FULL - 1):
                nc.tensor.matmul(
                    ad_ps, AIfull[:, j, :], wts_buf.pop(j),
                    start=False, stop=(j == TC_FULL - 1),
                )
            nc.scalar.copy(out=adaptT, in_=ad_ps)

            # ---------- finish output ----------
            def final_close(dc):
                op = ps_big.tile([128, T], F32, tag="mm")
                nc.tensor.matmul(
                    op, wfin[:, HC, dc * 128:(dc + 1) * 128], adaptT,
                    start=True, stop=False,
                )
                nc.tensor.matmul(
                    op, wfin[:, HC + 1, dc * 128:(dc + 1) * 128], mixedT,
                    start=False, stop=True,
                )
                ob = evac.tile([128, T], F32, tag="ob")
                nc.vector.tensor_tensor(
                    out=ob, in0=facc[:, dc, :], in1=op, op=mybir.AluOpType.add
                )
                nc.sync.dma_start(out=out_d[dc * 128:(dc + 1) * 128, :], in_=ob)

            for dc in range(DC):
                final_close(dc)

    nc.compile()
    return nc


def kernel(
    x, expert_weights, up_w, gate_w, down_w, pre_w, post_w, an_g, an_b,
    adapt_proj_w, adapter_w, adapter_g, adapter_b, expert_proj_w, output_proj_w,
):
    x = np.asarray(x, np.float32)
    expert_weights = np.asarray(expert_weights, np.float32)
    bf = ml_dtypes.bfloat16

    if "nc" not in _CACHE:
        _CACHE["nc"] = _build()
    nc = _CACHE["nc"]

    def pack(w, kc):
        # [kc*128, F] -> [128, kc, F] (partition-major SBUF layout)
        f = w.shape[1]
        return np.ascontiguousarray(
            w.reshape(kc, 128, f).transpose(1, 0, 2)
        ).astype(bf)

    ug_wT = np.concatenate(
        [np.asarray(up_w, np.float32), np.asarray(gate_w, np.float32)], axis=0
    ).T                                                        # [D, 2H]
    pre_wT = np.asarray(pre_w, np.float32).T                   # [D, A]
    post_pack = np.ascontiguousarray(
        np.asarray(post_w, np.float32).T.reshape(HC, 128, A).transpose(1, 2, 0)
    ).astype(bf)                                               # [128, A, HC]
    adapter_wT = (
        np.asarray(adapter_w, np.float32).transpose(2, 0, 1).reshape(A, E * A)
    ).astype(bf)                                               # [A, E*A] (e-major)
    down_w = np.asarray(down_w, np.float32)
    w_da = 0.1 * (down_w @ np.asarray(adapt_proj_w, np.float32))       # [D, A]
    w_mo = np.asarray(output_proj_w, np.float32) @ np.asarray(
        expert_proj_w, np.float32
    )                                                                   # [D, A]
    wfin = np.concatenate([down_w.T, w_da.T, w_mo.T], axis=0)  # [2304, D]
    angb = np.stack(
        [np.asarray(an_g, np.float32), np.asarray(an_b, np.float32)], axis=0
    )                                                                   # [2, A]
    ancol = np.ascontiguousarray(angb.T)                                # [A, 2]
    ag_row = np.asarray(adapter_g, np.float32).reshape(1, A * E).astype(bf)  # e-major
    bias_mix = (expert_weights @ np.asarray(adapter_b, np.float32)).astype(bf)

    xf = x.reshape(N, D)
    shared = {
        "ug_wT": pack(ug_wT, KD), "pre_wT": pack(pre_wT, KD),
        "post_wT": post_pack, "adapter_wT": adapter_wT,
        "wfin": pack(wfin, HC + 2), "angb": angb, "ancol": ancol,
        "ag_row": ag_row,
    }
    in_maps = []
    for c in range(NCORES):
        sl = slice(c * T, (c + 1) * T)
        ewc = np.ascontiguousarray(expert_weights[sl]).reshape(SC, 128, E)
        bmc = np.ascontiguousarray(bias_mix[sl]).reshape(SC, 128, A)
        in_maps.append(
            dict(
                shared,
                xT=pack(np.ascontiguousarray(xf[sl].T), KD),
                ew=np.ascontiguousarray(ewc.transpose(1, 0, 2)),
                bias_mix=np.ascontiguousarray(bmc.transpose(1, 0, 2)),
            )
        )

    try:
        res = run_bass_kernel_spmd(nc, in_maps, list(range(NCORES))).results
    except Exception:
        # axon workers occasionally hang up; one retry on a fresh dispatch
        import time

        time.sleep(10)
        res = run_bass_kernel_spmd(nc, in_maps, list(range(NCORES))).results
    out = np.empty((N, D), np.float32)
    for c in range(NCORES):
        out[c * T:(c + 1) * T] = res[c]["out"].T
    return out.reshape(B, S, D)



# revision 5
# speedup vs baseline: 1.3094x; 1.3094x over previous
"""Trainium2 Bass kernel for nn_ExpertGroup (moe_routing).

Contract: kernel(**inputs) takes FULL unsharded numpy inputs and returns the
FULL [2, 2048, 1024] fp32 output. Internally shards B*S=4096 tokens across
8 NeuronCores (512 tokens/core; cores 0-3 own batch 0, cores 4-7 batch 1),
replicates the small weights, and exchanges the per-batch adapter tensors
(adapt_in / adapt_out, [S,128] each) with two intra-group AllGathers.

All matmuls run in bf16 with fp32 PSUM accumulation. Activations are laid out
feature-major ([feature, token]) so every matmul contracts over partitions.

Host-side algebraic folds (exact, input-dependent, valid for any inputs):
  - up/gate weights concatenated into one [D, 2H] lhsT, packed hc-block-major
    so each 128-feature block is one contiguous DMA
  - shared_out + out = hidden @ down_w.T + adapt @ (0.1*down_w@adapt_proj_w).T
                      + mixed @ (output_proj_w@expert_proj_w).T
    -> mixed folded into the final_down PSUM accumulation; adapt closes it
  - sum_e ew[n,e]*adapter_b[e,:]  (LN bias term) = expert_weights @ adapter_b

Perf-critical structure (v2):
  - no DMA-xbar transposes (PE transpose via identity matmul) so the sync DMA
    queue never serializes behind collective-gated copies
  - AG1 is the first collective (~10us in) and absorbs the all-core entry
    barrier; AG2 is triggered right after the adapt_out LN with the expert
    path + final_down (~35us of PE) behind it to hide the mesh latency
  - xT split into per-chunk DMAs that land before the ug stream starts
"""

import sys

sys.path.insert(0, "/opt/trn_rl_repo")

import ml_dtypes
import numpy as np

import concourse.bass as bass
import concourse.mybir as mybir
import concourse.tile as tile
from concourse import bacc
from concourse.bass_utils import run_bass_kernel_spmd

BF16 = mybir.dt.bfloat16
F32 = mybir.dt.float32

B, S, D, E = 2, 2048, 1024, 8
H = 2 * D          # 2048
A = H // 16        # 128
N = B * S          # 4096
NCORES = 8
T = N // NCORES    # 512 tokens per core
GROUP = 4          # cores per batch
SC = T // 128      # 4 s-chunks per core
DC = D // 128      # 8 d-chunks (output features)
HC = H // 128      # 16 h-chunks
KD = D // 128      # 8 k-chunks over D
TC_FULL = S // 128  # 16 token-chunks per batch
EPS = 1e-5

_CACHE = {}


def _build():
    nc = bacc.Bacc(None, num_devices=NCORES)

    # ---- kernel I/O (per-core; weights pre-packed to SBUF layout on host) ----
    xT_d = nc.dram_tensor("xT", [128, KD, T], BF16, kind="ExternalInput")
    ug_d = nc.dram_tensor("ug_wT", [128, 2 * HC, KD, 128], BF16,
                          kind="ExternalInput")
    pre_d = nc.dram_tensor("pre_wT", [128, KD, A], BF16, kind="ExternalInput")
    post_d = nc.dram_tensor("post_wT", [128, A, HC], BF16, kind="ExternalInput")
    adw_d = nc.dram_tensor("adapter_wT", [A, E * A], BF16, kind="ExternalInput")
    wfin_d = nc.dram_tensor("wfin", [128, HC + 2, D], BF16, kind="ExternalInput")
    ew_d = nc.dram_tensor("ew", [128, SC, E], F32, kind="ExternalInput")
    angb_d = nc.dram_tensor("angb", [2, A], F32, kind="ExternalInput")
    ancol_d = nc.dram_tensor("ancol", [128, 2], F32, kind="ExternalInput")
    ag_d = nc.dram_tensor("ag_row", [1, A * E], BF16, kind="ExternalInput")
    bmix_d = nc.dram_tensor("bias_mix", [128, SC, A], BF16, kind="ExternalInput")
    ident_d = nc.dram_tensor("ident", [128, 128], BF16, kind="ExternalInput")
    out_d = nc.dram_tensor("out", [D, T], F32, kind="ExternalOutput")

    # ---- collective bounce buffers (internal DRAM) ----
    ag1_in = nc.dram_tensor("ag1_in", [T, A], BF16)
    ag1_out = nc.dram_tensor("ag1_out", [S, A], BF16)
    ag2_in = nc.dram_tensor("ag2_in", [A, T], BF16)
    ag2_out = nc.dram_tensor("ag2_out", [GROUP * A, T], BF16)
    RG = [[0, 1, 2, 3], [4, 5, 6, 7]]

    with tile.TileContext(nc) as tc:
        with (
            tc.tile_pool(name="consts", bufs=1) as consts,
            tc.tile_pool(name="wpool", bufs=1) as wpool,
            tc.tile_pool(name="acts", bufs=1) as acts,
            tc.tile_pool(name="work", bufs=4) as work,
            tc.tile_pool(name="work2", bufs=3) as work2,
            tc.tile_pool(name="wtp", bufs=4) as wtp,
            tc.tile_pool(name="workbig", bufs=1) as workbig,
            tc.tile_pool(name="aoln", bufs=1) as aoln,
            tc.tile_pool(name="evac", bufs=2) as evac,
            tc.tile_pool(name="ps_big", bufs=3, space="PSUM") as ps_big,
            tc.tile_pool(name="ps_poacc", bufs=1, space="PSUM") as ps_poacc,
            tc.tile_pool(name="ps_out", bufs=2, space="PSUM") as ps_out,
            tc.tile_pool(name="ps_sm", bufs=2, space="PSUM") as ps_sm,
        ):
            # ---------- priority loads: the pre-matmul chain needs these ----
            xT = wpool.tile([128, KD, T], BF16)
            for k in range(KD):
                nc.sync.dma_start(out=xT[:, k, :], in_=xT_d[:, k, :])
            pre_w = wpool.tile([128, KD, A], BF16)
            nc.sync.dma_start(out=pre_w, in_=pre_d[:])
            ident = consts.tile([128, 128], BF16)
            nc.sync.dma_start(out=ident, in_=ident_d[:])
            eps_t = consts.tile([128, 1], F32)
            nc.vector.memset(eps_t, EPS)
            gB = consts.tile([128, A], F32)   # an_g broadcast across partitions
            bB = consts.tile([128, A], F32)   # an_b broadcast
            nc.sync.dma_start(
                out=gB,
                in_=bass.AP(tensor=angb_d, offset=0, ap=[[0, 128], [1, A]]),
            )
            nc.sync.dma_start(
                out=bB,
                in_=bass.AP(tensor=angb_d, offset=A, ap=[[0, 128], [1, A]]),
            )

            # ---------- weight loads (block-granular so consumers start early)
            ug_w = wpool.tile([128, 2 * HC, KD, 128], BF16)
            for hc in range(HC):
                nc.sync.dma_start(out=ug_w[:, hc], in_=ug_d[:, hc])
                nc.sync.dma_start(out=ug_w[:, HC + hc], in_=ug_d[:, HC + hc])
            post_w = wpool.tile([128, A, HC], BF16)
            nc.sync.dma_start(out=post_w, in_=post_d[:])
            adw = wpool.tile([128, E * A], BF16)
            nc.sync.dma_start(out=adw, in_=adw_d[:])

            # ---------- remaining constants ----------
            agB = consts.tile([128, E, A], BF16)  # adapter_g (e-major) bcast
            nc.sync.dma_start(
                out=agB,
                in_=bass.AP(tensor=ag_d, offset=0, ap=[[0, 128], [A, E], [1, A]]),
            )
            ancol = consts.tile([128, 2], F32)
            nc.sync.dma_start(out=ancol, in_=ancol_d[:])
            ones_col = consts.tile([128, 1], BF16)
            nc.vector.memset(ones_col, 1.0)
            ones_row = consts.tile([1, 128], BF16)
            nc.vector.memset(ones_row, 1.0)
            ew_sb = consts.tile([128, SC, E], F32)
            nc.sync.dma_start(out=ew_sb, in_=ew_d[:])
            bmix_sb = consts.tile([128, SC, A], BF16)
            nc.sync.dma_start(out=bmix_sb, in_=bmix_d[:])
            wfin = wpool.tile([128, HC + 2, D], BF16)
            for k in range(HC + 2):
                nc.sync.dma_start(out=wfin[:, k, :], in_=wfin_d[:, k, :])

            # persistent activations
            AI_tok = acts.tile([128, SC, A], BF16)    # adapt_in, token-major
            AIT = acts.tile([128, T], BF16)           # adapt_in, feature-major
            AIfull = acts.tile([128, TC_FULL, A], BF16)  # gathered AI tok-major
            hid = acts.tile([128, HC, T], BF16)       # hidden, feature-major
            AOTfull = acts.tile([128, GROUP, T], BF16)   # gathered AO feat-major
            AOT = acts.tile([128, T], BF16)           # local AO, feature-major
            adaptT = acts.tile([128, T], BF16)        # adapt, feature-major
            mixedT = acts.tile([128, T], BF16)        # mixed, feature-major
            mix_tok = acts.tile([128, SC, A], BF16)   # mixed, token-major
            facc = acts.tile([128, DC, T], F32)       # down-part accumulator

            def layernorm_to(ps, dst):
                """LN over free dim (A=128) of psum tile [128, A]; write dst bf16."""
                st = work.tile([128, 6], F32, tag="lnst")
                nc.vector.bn_stats(out=st, in_=ps)
                mv = work.tile([128, 2], F32, tag="lnmv")
                nc.vector.bn_aggr(out=mv, in_=st)
                sd = work.tile([128, 1], F32, tag="lnsd")
                nc.scalar.activation(
                    out=sd, in_=mv[:, 1:2], func=mybir.ActivationFunctionType.Sqrt,
                    bias=eps_t, scale=1.0,
                )
                r = work.tile([128, 1], F32, tag="lnr")
                nc.vector.reciprocal(out=r, in_=sd)
                z = work.tile([128, A], F32, tag="lnz")
                nc.vector.tensor_scalar(
                    out=z, in0=ps, scalar1=mv[:, 0:1], scalar2=r,
                    op0=mybir.AluOpType.subtract, op1=mybir.AluOpType.mult,
                )
                zg = work.tile([128, A], F32, tag="lnzg")
                nc.vector.tensor_tensor(out=zg, in0=z, in1=gB, op=mybir.AluOpType.mult)
                nc.vector.tensor_tensor(out=dst, in0=zg, in1=bB, op=mybir.AluOpType.add)

            # ---------- adapt_in = LN(x @ pre_w.T) ----------
            for sc in range(SC):
                ps = ps_sm.tile([128, A], F32, tag="sm")
                for k in range(KD):
                    nc.tensor.matmul(
                        ps, xT[:, k, sc * 128:(sc + 1) * 128], pre_w[:, k, :],
                        start=(k == 0), stop=(k == KD - 1),
                    )
                layernorm_to(ps, AI_tok[:, sc, :])

            # local AI -> feature-major via PE transpose (no DMA xbar!)
            for sc in range(SC):
                tr = ps_sm.tile([128, 128], BF16, tag="sm")
                nc.tensor.transpose(tr, AI_tok[:, sc, :], ident)
                nc.scalar.copy(out=AIT[:, sc * 128:(sc + 1) * 128], in_=tr)

            # AllGather #1 (first collective: absorbs the all-core entry
            # barrier while the up/gate stream keeps the PE busy)
            nc.gpsimd.dma_start(
                out=ag1_in[:].rearrange("(sc p) a -> p sc a", p=128), in_=AI_tok
            )
            nc.gpsimd.collective_compute(
                "AllGather", mybir.AluOpType.bypass, replica_groups=RG,
                ins=[ag1_in[:]], outs=[ag1_out[:]],
            )
            nc.gpsimd.dma_start(
                out=AIfull, in_=ag1_out[:].rearrange("(k p) a -> p k a", p=128)
            )

            # ---------- hidden = silu(x@gate.T) * (x@up.T), feature-major -------
            # post contraction (adapt_out pre-LN, feature-major) interleaved with
            # one-iteration delay so PE never waits on the DVE mul.
            po_ps = ps_poacc.tile([128, T], F32, tag="po")

            def post_step(k):
                nc.tensor.matmul(
                    po_ps, post_w[:, :, k], hid[:, k, :],
                    start=(k == 0), stop=(k == HC - 1),
                )

            for hc in range(HC):
                up_ps = ps_big.tile([128, T], F32, tag="mm")
                gt_ps = ps_big.tile([128, T], F32, tag="mm")
                for k in range(KD):
                    nc.tensor.matmul(
                        up_ps, ug_w[:, hc, k, :], xT[:, k, :],
                        start=(k == 0), stop=(k == KD - 1),
                    )
                for k in range(KD):
                    nc.tensor.matmul(
                        gt_ps, ug_w[:, HC + hc, k, :], xT[:, k, :],
                        start=(k == 0), stop=(k == KD - 1),
                    )
                sg = work2.tile([128, T], BF16, tag="sg")
                nc.scalar.activation(
                    out=sg, in_=gt_ps, func=mybir.ActivationFunctionType.Silu
                )
                nc.vector.tensor_tensor(
                    out=hid[:, hc, :], in0=sg, in1=up_ps, op=mybir.AluOpType.mult
                )
                if hc > 0:
                    post_step(hc - 1)
            post_step(HC - 1)

            # ---------- adapt_out LN, feature-major (stats via PE ones-matmul) --
            AOf = acts.tile([128, T], BF16)
            nc.scalar.copy(out=AOf, in_=po_ps)
            sqf = aoln.tile([128, T], BF16)
            nc.vector.tensor_tensor(out=sqf, in0=AOf, in1=AOf, op=mybir.AluOpType.mult)
            s0 = ps_sm.tile([1, T], F32, tag="sm")
            nc.tensor.matmul(s0, ones_col, AOf, start=True, stop=True)
            s1 = ps_sm.tile([1, T], F32, tag="sm")
            nc.tensor.matmul(s1, ones_col, sqf, start=True, stop=True)
            mean_b = aoln.tile([1, T], BF16)
            nc.vector.tensor_scalar(
                out=mean_b, in0=s0, scalar1=1.0 / A, scalar2=None,
                op0=mybir.AluOpType.mult,
            )
            varf = aoln.tile([1, T], F32)
            nc.vector.tensor_scalar(
                out=varf, in0=s1, scalar1=1.0 / A, scalar2=None,
                op0=mybir.AluOpType.mult,
            )
            m2 = aoln.tile([1, T], F32, tag="fb")
            nc.vector.tensor_tensor(out=m2, in0=mean_b, in1=mean_b,
                                    op=mybir.AluOpType.mult)
            nc.vector.tensor_tensor(out=varf, in0=varf, in1=m2,
                                    op=mybir.AluOpType.subtract)
            sdf = aoln.tile([1, T], F32, tag="fa")
            nc.scalar.activation(
                out=sdf, in_=varf, func=mybir.ActivationFunctionType.Sqrt,
                bias=eps_t[0:1], scale=1.0,
            )
            rstd_f = aoln.tile([1, T], F32, tag="fb")
            nc.vector.reciprocal(out=rstd_f, in_=sdf)
            rstd_b = aoln.tile([1, T], BF16)
            nc.vector.tensor_copy(out=rstd_b, in_=rstd_f)
            meanB = ps_sm.tile([128, T], F32, tag="sm")
            nc.tensor.matmul(meanB, ones_row, mean_b, start=True, stop=True)
            rstdB = ps_sm.tile([128, T], F32, tag="sm")
            nc.tensor.matmul(rstdB, ones_row, rstd_b, start=True, stop=True)
            z1 = aoln.tile([128, T], BF16)
            nc.vector.tensor_tensor(out=z1, in0=AOf, in1=meanB,
                                    op=mybir.AluOpType.subtract)
            z2 = aoln.tile([128, T], BF16)
            nc.vector.tensor_tensor(out=z2, in0=z1, in1=rstdB,
                                    op=mybir.AluOpType.mult)
            nc.vector.tensor_scalar(
                out=AOT, in0=z2, scalar1=ancol[:, 0:1], scalar2=ancol[:, 1:2],
                op0=mybir.AluOpType.mult, op1=mybir.AluOpType.add,
            )
            nc.gpsimd.dma_start(out=ag2_in[:], in_=AOT)
            nc.gpsimd.collective_compute(
                "AllGather", mybir.AluOpType.bypass, replica_groups=RG,
                ins=[ag2_in[:]], outs=[ag2_out[:]],
            )
            # gathered AO -> [A, c, T] in one strided DMA
            nc.gpsimd.dma_start(
                out=AOTfull,
                in_=bass.AP(tensor=ag2_out, offset=0,
                            ap=[[T, 128], [128 * T, GROUP], [1, T]]),
            )
            AOTf = AOTfull.rearrange("a c t -> a (c t)")

            # ---------- expert path (local tokens only; fills the AG2 window) ---
            for sc in range(SC):
                hp0 = ps_big.tile([128, 512], F32, tag="mm")
                hp1 = ps_big.tile([128, 512], F32, tag="mm")
                sl = AIT[:, sc * 128:(sc + 1) * 128]
                nc.tensor.matmul(hp0, sl, adw[:, 0:512], start=True, stop=True)
                nc.tensor.matmul(hp1, sl, adw[:, 512:1024], start=True, stop=True)
                hps = [hp0, hp0, hp0, hp0, hp1, hp1, hp1, hp1]
                st8 = work.tile([128, E, 6], F32, tag="st8")
                for e in range(E):
                    nc.vector.bn_stats(
                        out=st8[:, e, :], in_=hps[e][:, (e % 4) * A:(e % 4 + 1) * A]
                    )
                mv8 = work.tile([128, E, 2], F32, tag="mv8")
                for e in range(E):
                    nc.vector.bn_aggr(out=mv8[:, e, :], in_=st8[:, e, :])
                sd8 = work.tile([128, E], F32, tag="sd8")
                nc.scalar.activation(
                    out=sd8, in_=mv8[:, :, 1], func=mybir.ActivationFunctionType.Sqrt,
                    bias=eps_t, scale=1.0,
                )
                r8 = work.tile([128, E], F32, tag="r8")
                nc.vector.reciprocal(out=r8, in_=sd8)
                rw8 = work.tile([128, E], F32, tag="rw8")
                nc.vector.tensor_tensor(
                    out=rw8, in0=r8, in1=ew_sb[:, sc, :], op=mybir.AluOpType.mult
                )
                nmrw = work.tile([128, E], F32, tag="nmrw")
                nc.vector.tensor_tensor(
                    out=nmrw, in0=mv8[:, :, 0], in1=rw8, op=mybir.AluOpType.mult
                )
                nc.vector.tensor_scalar(
                    out=nmrw, in0=nmrw, scalar1=-1.0, scalar2=None,
                    op0=mybir.AluOpType.mult,
                )
                # z~_e = h_e * (r*ew)_e - m*(r*ew)_e, written e-outer [s, e, c]
                zt = workbig.tile([128, E, A], BF16, tag="zt")
                for e in range(E):
                    nc.scalar.activation(
                        out=zt[:, e, :], in_=hps[e][:, (e % 4) * A:(e % 4 + 1) * A],
                        func=mybir.ActivationFunctionType.Identity,
                        scale=rw8[:, e:e + 1], bias=nmrw[:, e:e + 1],
                    )
                zg = workbig.tile([128, E, A], BF16, tag="ztg")
                nc.vector.tensor_tensor(
                    out=zg, in0=zt, in1=agB, op=mybir.AluOpType.mult
                )
                t1 = workbig.tile([128, 4, A], BF16, tag="sum1")
                nc.vector.tensor_tensor(
                    out=t1, in0=zg[:, 0:4, :], in1=zg[:, 4:8, :],
                    op=mybir.AluOpType.add,
                )
                t2 = work.tile([128, 2, A], BF16, tag="sum2")
                nc.vector.tensor_tensor(
                    out=t2, in0=t1[:, 0:2, :], in1=t1[:, 2:4, :],
                    op=mybir.AluOpType.add,
                )
                mx = work.tile([128, A], BF16, tag="mx")
                nc.vector.tensor_tensor(
                    out=mx, in0=t2[:, 0, :], in1=t2[:, 1, :], op=mybir.AluOpType.add
                )
                nc.vector.tensor_tensor(
                    out=mix_tok[:, sc, :], in0=mx, in1=bmix_sb[:, sc, :],
                    op=mybir.AluOpType.add,
                )
            # mixed -> feature-major via PE transpose
            for sc in range(SC):
                tr = ps_sm.tile([128, 128], BF16, tag="sm")
                nc.tensor.transpose(tr, mix_tok[:, sc, :], ident)
                nc.scalar.copy(out=mixedT[:, sc * 128:(sc + 1) * 128], in_=tr)

            # ---------- final output down-part + mixed-part (AG2 window) -------
            def final_down(dc):
                op = ps_out.tile([128, T], F32, tag="fout")
                for k in range(HC):
                    nc.tensor.matmul(
                        op, wfin[:, k, dc * 128:(dc + 1) * 128], hid[:, k, :],
                        start=(k == 0), stop=False,
                    )
                nc.tensor.matmul(
                    op, wfin[:, HC + 1, dc * 128:(dc + 1) * 128], mixedT,
                    start=False, stop=True,
                )
                nc.scalar.copy(out=facc[:, dc, :], in_=op)

            for dc in range(DC):
                final_down(dc)

            # ---------- w = silu(clip(AI_loc @ AO_full.T)); adapt = w.T chain ---
            ad_ps = ps_poacc.tile([128, T], F32, tag="po")
            wts_buf = {}

            def bmm1_step(j):
                w_ps = ps_big.tile([128, T], F32, tag="mm")
                nc.tensor.matmul(
                    w_ps, AOTf[:, j * 128:(j + 1) * 128], AIT, start=True, stop=True
                )
                wc = work2.tile([128, T], BF16, tag="wc")
                nc.vector.tensor_scalar(
                    out=wc, in0=w_ps, scalar1=-5.0, scalar2=5.0,
                    op0=mybir.AluOpType.max, op1=mybir.AluOpType.min,
                )
                wt = wtp.tile([128, T], BF16, tag="wts")
                nc.scalar.activation(
                    out=wt, in_=wc, func=mybir.ActivationFunctionType.Silu
                )
                wts_buf[j] = wt

            # depth-3 software pipeline: bmm2(j) trails bmm1(j+3) so the PE
            # never waits on the clip/silu stages
            DEPTH = 3
            for j in range(TC_FULL):
                bmm1_step(j)
                if j >= DEPTH:
                    nc.tensor.matmul(
                        ad_ps, AIfull[:, j - DEPTH, :], wts_buf.pop(j - DEPTH),
                        start=(j - DEPTH == 0), stop=False,
                    )
            for j in range(TC_FULL - DEPTH, TC_FULL):
                nc.tensor.matmul(
                    ad_ps, AIfull[:, j, :], wts_buf.pop(j),
                    start=False, stop=(j == TC_FULL - 1),
                )
            nc.scalar.copy(out=adaptT, in_=ad_ps)

            # ---------- finish output ----------
            def final_close(dc):
                op = ps_big.tile([128, T], F32, tag="mm")
                nc.tensor.matmul(
                    op, wfin[:, HC, dc * 128:(dc + 1) * 128], adaptT,
                    start=True, stop=True,
                )
                ob = evac.tile([128, T], F32, tag="ob")
                nc.vector.tensor_tensor(
                    out=ob, in0=facc[:, dc, :], in1=op, op=mybir.AluOpType.add
                )
                nc.sync.dma_start(out=out_d[dc * 128:(dc + 1) * 128, :], in_=ob)

            for dc in range(DC):
                final_close(dc)

    nc.compile()
    return nc


def kernel(
    x, expert_weights, up_w, gate_w, down_w, pre_w, post_w, an_g, an_b,
    adapt_proj_w, adapter_w, adapter_g, adapter_b, expert_proj_w, output_proj_w,
):
    x = np.asarray(x, np.float32)
    expert_weights = np.asarray(expert_weights, np.float32)
    bf = ml_dtypes.bfloat16

    if "nc" not in _CACHE:
        _CACHE["nc"] = _build()
    nc = _CACHE["nc"]

    def pack(w, kc):
        # [kc*128, F] -> [128, kc, F] (partition-major SBUF layout)
        f = w.shape[1]
        return np.ascontiguousarray(
            w.reshape(kc, 128, f).transpose(1, 0, 2)
        ).astype(bf)

    ug_wT = np.concatenate(
        [np.asarray(up_w, np.float32), np.asarray(gate_w, np.float32)], axis=0
    ).T                                                        # [D, 2H]
    # hc-block-major: [128, 2*HC blocks, KD, 128]
    ug_pack = np.ascontiguousarray(
        ug_wT.reshape(KD, 128, 2 * HC, 128).transpose(1, 2, 0, 3)
    ).astype(bf)
    pre_wT = np.asarray(pre_w, np.float32).T                   # [D, A]
    post_pack = np.ascontiguousarray(
        np.asarray(post_w, np.float32).T.reshape(HC, 128, A).transpose(1, 2, 0)
    ).astype(bf)                                               # [128, A, HC]
    adapter_wT = (
        np.asarray(adapter_w, np.float32).transpose(2, 0, 1).reshape(A, E * A)
    ).astype(bf)                                               # [A, E*A] (e-major)
    down_w = np.asarray(down_w, np.float32)
    w_da = 0.1 * (down_w @ np.asarray(adapt_proj_w, np.float32))       # [D, A]
    w_mo = np.asarray(output_proj_w, np.float32) @ np.asarray(
        expert_proj_w, np.float32
    )                                                                   # [D, A]
    wfin = np.concatenate([down_w.T, w_da.T, w_mo.T], axis=0)  # [2304, D]
    angb = np.stack(
        [np.asarray(an_g, np.float32), np.asarray(an_b, np.float32)], axis=0
    )                                                                   # [2, A]
    ancol = np.ascontiguousarray(angb.T)                                # [A, 2]
    ag_row = np.asarray(adapter_g, np.float32).reshape(1, A * E).astype(bf)  # e-major
    bias_mix = (expert_weights @ np.asarray(adapter_b, np.float32)).astype(bf)

    xf = x.reshape(N, D)
    shared = {
        "ug_wT": ug_pack, "pre_wT": pack(pre_wT, KD),
        "post_wT": post_pack, "adapter_wT": adapter_wT,
        "wfin": pack(wfin, HC + 2), "angb": angb, "ancol": ancol,
        "ag_row": ag_row, "ident": np.eye(128, dtype=bf),
    }
    in_maps = []
    for c in range(NCORES):
        sl = slice(c * T, (c + 1) * T)
        ewc = np.ascontiguousarray(expert_weights[sl]).reshape(SC, 128, E)
        bmc = np.ascontiguousarray(bias_mix[sl]).reshape(SC, 128, A)
        in_maps.append(
            dict(
                shared,
                xT=pack(np.ascontiguousarray(xf[sl].T), KD),
                ew=np.ascontiguousarray(ewc.transpose(1, 0, 2)),
                bias_mix=np.ascontiguousarray(bmc.transpose(1, 0, 2)),
            )
        )

    try:
        res = run_bass_kernel_spmd(nc, in_maps, list(range(NCORES))).results
    except Exception:
        # axon workers occasionally hang up; one retry on a fresh dispatch
        import time

        time.sleep(10)
        res = run_bass_kernel_spmd(nc, in_maps, list(range(NCORES))).results
    out = np.empty((N, D), np.float32)
    for c in range(NCORES):
        out[c * T:(c + 1) * T] = res[c]["out"].T
    return out.reshape(B, S, D)


# revision 9
# speedup vs baseline: 1.5318x; 1.1699x over previous
"""Trainium2 Bass kernel for nn_ExpertGroup (moe_routing).

Contract: kernel(**inputs) takes FULL unsharded numpy inputs and returns the
FULL [2, 2048, 1024] fp32 output. Internally shards B*S=4096 tokens across
8 NeuronCores (512 tokens/core; cores 0-3 own batch 0, cores 4-7 batch 1),
replicates the small weights, and exchanges the per-batch adapter tensors
(adapt_in / adapt_out, [S,128] each) with two intra-group AllGathers.

All matmuls run in bf16 with fp32 PSUM accumulation. Activations are laid out
feature-major ([feature, token]) so every matmul contracts over partitions.

Host-side algebraic folds (exact, input-dependent, valid for any inputs):
  - up/gate weights concatenated into one [D, 2H] lhsT, packed hc-block-major
    so each 128-feature block is one contiguous DMA
  - shared_out + out = hidden @ down_w.T + adapt @ (0.1*down_w@adapt_proj_w).T
                      + mixed @ (output_proj_w@expert_proj_w).T
    -> mixed folded into the final_down PSUM accumulation; adapt closes it
  - sum_e ew[n,e]*adapter_b[e,:]  (LN bias term) = expert_weights @ adapter_b

Perf-critical structure (v2):
  - no DMA-xbar transposes (PE transpose via identity matmul) so the sync DMA
    queue never serializes behind collective-gated copies
  - AG1 is the first collective (~10us in) and absorbs the all-core entry
    barrier; AG2 is triggered right after the adapt_out LN with the expert
    path + final_down (~35us of PE) behind it to hide the mesh latency
  - xT split into per-chunk DMAs that land before the ug stream starts
"""

import sys

sys.path.insert(0, "/opt/trn_rl_repo")

import ml_dtypes
import numpy as np

import concourse.bass as bass
import concourse.mybir as mybir
import concourse.tile as tile
from concourse import bacc
from concourse.bass_utils import run_bass_kernel_spmd

BF16 = mybir.dt.bfloat16
F32 = mybir.dt.float32

B, S, D, E = 2, 2048, 1024, 8
H = 2 * D          # 2048
A = H // 16        # 128
N = B * S          # 4096
NCORES = 8
T = N // NCORES    # 512 tokens per core
GROUP = 4          # cores per batch
SC = T // 128      # 4 s-chunks per core
DC = D // 128      # 8 d-chunks (output features)
HC = H // 128      # 16 h-chunks
KD = D // 128      # 8 k-chunks over D
TC_FULL = S // 128  # 16 token-chunks per batch
EPS = 1e-5

_CACHE = {}


def _build():
    nc = bacc.Bacc(None, num_devices=NCORES)

    # ---- kernel I/O (per-core; weights pre-packed to SBUF layout on host) ----
    xT_d = nc.dram_tensor("xT", [128, KD, T], BF16, kind="ExternalInput")
    ug_d = nc.dram_tensor("ug_wT", [128, 2 * HC, KD, 128], BF16,
                          kind="ExternalInput")
    pre_d = nc.dram_tensor("pre_wT", [128, KD, A], BF16, kind="ExternalInput")
    post_d = nc.dram_tensor("post_wT", [128, A, HC], BF16, kind="ExternalInput")
    adw_d = nc.dram_tensor("adapter_wT", [A, E * A], BF16, kind="ExternalInput")
    wfin_d = nc.dram_tensor("wfin", [128, HC + 2, D], BF16, kind="ExternalInput")
    ew_d = nc.dram_tensor("ew", [128, SC, E], F32, kind="ExternalInput")
    angb_d = nc.dram_tensor("angb", [2, A], F32, kind="ExternalInput")
    ancol_d = nc.dram_tensor("ancol", [128, 2], F32, kind="ExternalInput")
    ag_d = nc.dram_tensor("ag_row", [1, A * E], BF16, kind="ExternalInput")
    bmix_d = nc.dram_tensor("bias_mix", [128, SC, A], BF16, kind="ExternalInput")
    ident_d = nc.dram_tensor("ident", [128, 128], BF16, kind="ExternalInput")
    out_d = nc.dram_tensor("out", [D, T], F32, kind="ExternalOutput")

    # ---- collective bounce buffers (internal DRAM) ----
    ag1_in = nc.dram_tensor("ag1_in", [T, A], BF16)
    ag1_out = nc.dram_tensor("ag1_out", [S, A], BF16)
    ag2_in = nc.dram_tensor("ag2_in", [A, T], BF16)
    ag2_out = nc.dram_tensor("ag2_out", [GROUP * A, T], BF16)
    RG = [[0, 1, 2, 3], [4, 5, 6, 7]]

    with tile.TileContext(nc) as tc:
        with (
            tc.tile_pool(name="consts", bufs=1) as consts,
            tc.tile_pool(name="wpool", bufs=1) as wpool,
            tc.tile_pool(name="acts", bufs=1) as acts,
            tc.tile_pool(name="work", bufs=4) as work,
            tc.tile_pool(name="work2", bufs=3) as work2,
            tc.tile_pool(name="wtp", bufs=4) as wtp,
            tc.tile_pool(name="workbig", bufs=1) as workbig,
            tc.tile_pool(name="aoln", bufs=1) as aoln,
            tc.tile_pool(name="evac", bufs=2) as evac,
            tc.tile_pool(name="ps_big", bufs=3, space="PSUM") as ps_big,
            tc.tile_pool(name="ps_poacc", bufs=1, space="PSUM") as ps_poacc,
            tc.tile_pool(name="ps_out", bufs=2, space="PSUM") as ps_out,
            tc.tile_pool(name="ps_sm", bufs=2, space="PSUM") as ps_sm,
        ):
            # ---------- priority loads: the pre-matmul chain needs these ----
            # sync queue carries the big streams (xT then ug blocks); the tiny
            # early tensors ride the scalar queue so ug issue isn't delayed.
            xT = wpool.tile([128, KD, T], BF16)
            nc.sync.dma_start(out=xT, in_=xT_d[:])
            pre_w = wpool.tile([128, KD, A], BF16)
            nc.scalar.dma_start(out=pre_w, in_=pre_d[:])
            ident = consts.tile([128, 128], BF16)
            nc.scalar.dma_start(out=ident, in_=ident_d[:])
            eps_t = consts.tile([128, 1], F32)
            nc.vector.memset(eps_t, EPS)
            gB = consts.tile([128, A], F32)   # an_g broadcast across partitions
            bB = consts.tile([128, A], F32)   # an_b broadcast
            nc.scalar.dma_start(
                out=gB,
                in_=bass.AP(tensor=angb_d, offset=0, ap=[[0, 128], [1, A]]),
            )
            nc.scalar.dma_start(
                out=bB,
                in_=bass.AP(tensor=angb_d, offset=A, ap=[[0, 128], [1, A]]),
            )

            # ---------- weight loads (block-granular so consumers start early)
            ug_w = wpool.tile([128, 2 * HC, KD, 128], BF16)
            for hc in range(HC):
                nc.sync.dma_start(out=ug_w[:, hc], in_=ug_d[:, hc])
                nc.sync.dma_start(out=ug_w[:, HC + hc], in_=ug_d[:, HC + hc])
            post_w = wpool.tile([128, A, HC], BF16)
            nc.sync.dma_start(out=post_w, in_=post_d[:])
            adw = wpool.tile([128, E * A], BF16)
            nc.sync.dma_start(out=adw, in_=adw_d[:])

            # ---------- remaining constants ----------
            agB = consts.tile([128, E, A], BF16)  # adapter_g (e-major) bcast
            nc.scalar.dma_start(
                out=agB,
                in_=bass.AP(tensor=ag_d, offset=0, ap=[[0, 128], [A, E], [1, A]]),
            )
            ancol = consts.tile([128, 2], F32)
            nc.scalar.dma_start(out=ancol, in_=ancol_d[:])
            ones_col = consts.tile([128, 1], BF16)
            nc.vector.memset(ones_col, 1.0)
            ones_row = consts.tile([1, 128], BF16)
            nc.vector.memset(ones_row, 1.0)
            ew_sb = consts.tile([128, SC, E], F32)
            nc.sync.dma_start(out=ew_sb, in_=ew_d[:])
            bmix_sb = consts.tile([128, SC, A], BF16)
            nc.sync.dma_start(out=bmix_sb, in_=bmix_d[:])
            wfin = wpool.tile([128, HC + 2, D], BF16)
            for k in range(HC + 2):
                nc.sync.dma_start(out=wfin[:, k, :], in_=wfin_d[:, k, :])

            # persistent activations
            AI_tok = acts.tile([128, SC, A], BF16)    # adapt_in, token-major
            AIT = acts.tile([128, T], BF16)           # adapt_in, feature-major
            AIfull = acts.tile([128, TC_FULL, A], BF16)  # gathered AI tok-major
            hid = acts.tile([128, HC, T], BF16)       # hidden, feature-major
            AOTfull = acts.tile([128, GROUP, T], BF16)   # gathered AO feat-major
            AOT = acts.tile([128, T], BF16)           # local AO, feature-major
            adaptT = acts.tile([128, T], BF16)        # adapt, feature-major
            mixedT = acts.tile([128, T], BF16)        # mixed, feature-major
            mix_tok = acts.tile([128, SC, A], BF16)   # mixed, token-major
            facc = acts.tile([128, DC, T], F32)       # down-part accumulator

            def layernorm_to(ps, dst):
                """LN over free dim (A=128) of psum tile [128, A]; write dst bf16."""
                st = work.tile([128, 6], F32, tag="lnst")
                nc.vector.bn_stats(out=st, in_=ps)
                mv = work.tile([128, 2], F32, tag="lnmv")
                nc.vector.bn_aggr(out=mv, in_=st)
                sd = work.tile([128, 1], F32, tag="lnsd")
                nc.scalar.activation(
                    out=sd, in_=mv[:, 1:2], func=mybir.ActivationFunctionType.Sqrt,
                    bias=eps_t, scale=1.0,
                )
                r = work.tile([128, 1], F32, tag="lnr")
                nc.vector.reciprocal(out=r, in_=sd)
                z = work.tile([128, A], F32, tag="lnz")
                nc.vector.tensor_scalar(
                    out=z, in0=ps, scalar1=mv[:, 0:1], scalar2=r,
                    op0=mybir.AluOpType.subtract, op1=mybir.AluOpType.mult,
                )
                zg = work.tile([128, A], F32, tag="lnzg")
                nc.vector.tensor_tensor(out=zg, in0=z, in1=gB, op=mybir.AluOpType.mult)
                nc.vector.tensor_tensor(out=dst, in0=zg, in1=bB, op=mybir.AluOpType.add)

            # ---------- adapt_in = LN(x @ pre_w.T) ----------
            for sc in range(SC):
                ps = ps_sm.tile([128, A], F32, tag="sm")
                for k in range(KD):
                    nc.tensor.matmul(
                        ps, xT[:, k, sc * 128:(sc + 1) * 128], pre_w[:, k, :],
                        start=(k == 0), stop=(k == KD - 1),
                    )
                layernorm_to(ps, AI_tok[:, sc, :])

            # local AI -> feature-major via PE transpose (no DMA xbar!)
            for sc in range(SC):
                tr = ps_sm.tile([128, 128], BF16, tag="sm")
                nc.tensor.transpose(tr, AI_tok[:, sc, :], ident)
                nc.scalar.copy(out=AIT[:, sc * 128:(sc + 1) * 128], in_=tr)

            # AllGather #1 (first collective: absorbs the all-core entry
            # barrier while the up/gate stream keeps the PE busy)
            nc.gpsimd.dma_start(
                out=ag1_in[:].rearrange("(sc p) a -> p sc a", p=128), in_=AI_tok
            )
            nc.gpsimd.collective_compute(
                "AllGather", mybir.AluOpType.bypass, replica_groups=RG,
                ins=[ag1_in[:]], outs=[ag1_out[:]],
            )
            nc.gpsimd.dma_start(
                out=AIfull, in_=ag1_out[:].rearrange("(k p) a -> p k a", p=128)
            )

            # ---------- hidden = silu(x@gate.T) * (x@up.T), feature-major -------
            # post contraction (adapt_out pre-LN, feature-major) interleaved with
            # one-iteration delay so PE never waits on the DVE mul.
            po_ps = ps_poacc.tile([128, T], F32, tag="po")

            def post_step(k):
                nc.tensor.matmul(
                    po_ps, post_w[:, :, k], hid[:, k, :],
                    start=(k == 0), stop=(k == HC - 1),
                )

            for hc in range(HC):
                up_ps = ps_big.tile([128, T], F32, tag="mm")
                gt_ps = ps_big.tile([128, T], F32, tag="mm")
                for k in range(KD):
                    nc.tensor.matmul(
                        up_ps, ug_w[:, hc, k, :], xT[:, k, :],
                        start=(k == 0), stop=(k == KD - 1),
                    )
                for k in range(KD):
                    nc.tensor.matmul(
                        gt_ps, ug_w[:, HC + hc, k, :], xT[:, k, :],
                        start=(k == 0), stop=(k == KD - 1),
                    )
                sg = work2.tile([128, T], BF16, tag="sg")
                nc.scalar.activation(
                    out=sg, in_=gt_ps, func=mybir.ActivationFunctionType.Silu
                )
                nc.vector.tensor_tensor(
                    out=hid[:, hc, :], in0=sg, in1=up_ps, op=mybir.AluOpType.mult
                )
                if hc > 0:
                    post_step(hc - 1)
            post_step(HC - 1)

            # ---------- adapt_out LN, feature-major (stats via PE ones-matmul) --
            # The PE pieces of the LN chain are interleaved with expert-path
            # matmuls and the first final_down chunks so the PE never idles
            # while DVE/ACT walk the stat chain; the DVE ops that feed the AG2
            # trigger come before any expert DVE work (queue priority).
            AOf = acts.tile([128, T], BF16)
            nc.scalar.copy(out=AOf, in_=po_ps)
            sqf = aoln.tile([128, T], BF16)
            nc.vector.tensor_tensor(out=sqf, in0=AOf, in1=AOf, op=mybir.AluOpType.mult)
            s0 = ps_sm.tile([1, T], F32, tag="sm")
            nc.tensor.matmul(s0, ones_col, AOf, start=True, stop=True)
            s1 = ps_sm.tile([1, T], F32, tag="sm")
            nc.tensor.matmul(s1, ones_col, sqf, start=True, stop=True)

            hp_buf = {}

            def expert_mms(sc):
                hp0 = ps_big.tile([128, 512], F32, tag="mm")
                hp1 = ps_big.tile([128, 512], F32, tag="mm")
                sl = AIT[:, sc * 128:(sc + 1) * 128]
                nc.tensor.matmul(hp0, sl, adw[:, 0:512], start=True, stop=True)
                nc.tensor.matmul(hp1, sl, adw[:, 512:1024], start=True, stop=True)
                hp_buf[sc] = (hp0, hp1)

            expert_mms(0)

            mean_b = aoln.tile([1, T], BF16)
            nc.vector.tensor_scalar(
                out=mean_b, in0=s0, scalar1=1.0 / A, scalar2=None,
                op0=mybir.AluOpType.mult,
            )
            varf = aoln.tile([1, T], F32)
            nc.vector.tensor_scalar(
                out=varf, in0=s1, scalar1=1.0 / A, scalar2=None,
                op0=mybir.AluOpType.mult,
            )
            m2 = aoln.tile([1, T], F32, tag="fb")
            nc.vector.tensor_tensor(out=m2, in0=mean_b, in1=mean_b,
                                    op=mybir.AluOpType.mult)
            nc.vector.tensor_tensor(out=varf, in0=varf, in1=m2,
                                    op=mybir.AluOpType.subtract)
            sdf = aoln.tile([1, T], F32, tag="fa")
            nc.scalar.activation(
                out=sdf, in_=varf, func=mybir.ActivationFunctionType.Sqrt,
                bias=eps_t[0:1], scale=1.0,
            )
            rstd_f = aoln.tile([1, T], F32, tag="fb")
            nc.vector.reciprocal(out=rstd_f, in_=sdf)
            rstd_b = aoln.tile([1, T], BF16)
            nc.vector.tensor_copy(out=rstd_b, in_=rstd_f)
            meanB = ps_sm.tile([128, T], F32, tag="sm")
            nc.tensor.matmul(meanB, ones_row, mean_b, start=True, stop=True)
            rstdB = ps_sm.tile([128, T], F32, tag="sm")
            nc.tensor.matmul(rstdB, ones_row, rstd_b, start=True, stop=True)

            # ---------- final output down-part (fills the AG2 window) -------
            def final_down(dc):
                op = ps_out.tile([128, T], F32, tag="fout")
                for k in range(HC):
                    nc.tensor.matmul(
                        op, wfin[:, k, dc * 128:(dc + 1) * 128], hid[:, k, :],
                        start=(k == 0), stop=(k == HC - 1),
                    )
                nc.scalar.copy(out=facc[:, dc, :], in_=op)

            final_down(0)

            z1 = aoln.tile([128, T], BF16)
            nc.vector.tensor_tensor(out=z1, in0=AOf, in1=meanB,
                                    op=mybir.AluOpType.subtract)
            z2 = aoln.tile([128, T], BF16)
            nc.vector.tensor_tensor(out=z2, in0=z1, in1=rstdB,
                                    op=mybir.AluOpType.mult)
            nc.vector.tensor_scalar(
                out=AOT, in0=z2, scalar1=ancol[:, 0:1], scalar2=ancol[:, 1:2],
                op0=mybir.AluOpType.mult, op1=mybir.AluOpType.add,
            )
            nc.gpsimd.dma_start(out=ag2_in[:], in_=AOT)
            nc.gpsimd.collective_compute(
                "AllGather", mybir.AluOpType.bypass, replica_groups=RG,
                ins=[ag2_in[:]], outs=[ag2_out[:]],
            )
            # gathered AO -> [A, c, T] in one strided DMA
            nc.gpsimd.dma_start(
                out=AOTfull,
                in_=bass.AP(tensor=ag2_out, offset=0,
                            ap=[[T, 128], [128 * T, GROUP], [1, T]]),
            )
            AOTf = AOTfull.rearrange("a c t -> a (c t)")

            # ---------- expert path LN/mix (DVE/ACT; overlaps final_down) ---
            def expert_rest(sc):
                hp0, hp1 = hp_buf.pop(sc)
                hps = [hp0, hp0, hp0, hp0, hp1, hp1, hp1, hp1]
                st8 = work.tile([128, E, 6], F32, tag="st8")
                for e in range(E):
                    nc.vector.bn_stats(
                        out=st8[:, e, :], in_=hps[e][:, (e % 4) * A:(e % 4 + 1) * A]
                    )
                mv8 = work.tile([128, E, 2], F32, tag="mv8")
                for e in range(E):
                    nc.vector.bn_aggr(out=mv8[:, e, :], in_=st8[:, e, :])
                sd8 = work.tile([128, E], F32, tag="sd8")
                nc.scalar.activation(
                    out=sd8, in_=mv8[:, :, 1], func=mybir.ActivationFunctionType.Sqrt,
                    bias=eps_t, scale=1.0,
                )
                r8 = work.tile([128, E], F32, tag="r8")
                nc.vector.reciprocal(out=r8, in_=sd8)
                rw8 = work.tile([128, E], F32, tag="rw8")
                nc.vector.tensor_tensor(
                    out=rw8, in0=r8, in1=ew_sb[:, sc, :], op=mybir.AluOpType.mult
                )
                nmrw = work.tile([128, E], F32, tag="nmrw")
                nc.vector.tensor_tensor(
                    out=nmrw, in0=mv8[:, :, 0], in1=rw8, op=mybir.AluOpType.mult
                )
                nc.vector.tensor_scalar(
                    out=nmrw, in0=nmrw, scalar1=-1.0, scalar2=None,
                    op0=mybir.AluOpType.mult,
                )
                # z~_e = h_e * (r*ew)_e - m*(r*ew)_e, written e-outer [s, e, c]
                zt = workbig.tile([128, E, A], BF16, tag="zt")
                for e in range(E):
                    nc.scalar.activation(
                        out=zt[:, e, :], in_=hps[e][:, (e % 4) * A:(e % 4 + 1) * A],
                        func=mybir.ActivationFunctionType.Identity,
                        scale=rw8[:, e:e + 1], bias=nmrw[:, e:e + 1],
                    )
                zg = workbig.tile([128, E, A], BF16, tag="ztg")
                nc.vector.tensor_tensor(
                    out=zg, in0=zt, in1=agB, op=mybir.AluOpType.mult
                )
                t1 = workbig.tile([128, 4, A], BF16, tag="sum1")
                nc.vector.tensor_tensor(
                    out=t1, in0=zg[:, 0:4, :], in1=zg[:, 4:8, :],
                    op=mybir.AluOpType.add,
                )
                t2 = work.tile([128, 2, A], BF16, tag="sum2")
                nc.vector.tensor_tensor(
                    out=t2, in0=t1[:, 0:2, :], in1=t1[:, 2:4, :],
                    op=mybir.AluOpType.add,
                )
                mx = work.tile([128, A], BF16, tag="mx")
                nc.vector.tensor_tensor(
                    out=mx, in0=t2[:, 0, :], in1=t2[:, 1, :], op=mybir.AluOpType.add
                )
                nc.vector.tensor_tensor(
                    out=mix_tok[:, sc, :], in0=mx, in1=bmix_sb[:, sc, :],
                    op=mybir.AluOpType.add,
                )

            # interleave: expert matmuls + remaining down chunks keep the PE
            # dense while DVE/ACT walk the expert LN chains concurrently
            expert_rest(0)
            expert_mms(1)
            final_down(1)
            expert_rest(1)
            expert_mms(2)
            final_down(2)
            expert_rest(2)
            expert_mms(3)
            final_down(3)
            expert_rest(3)
            for dc in range(4, DC):
                final_down(dc)

            # mixed -> feature-major via PE transpose
            for sc in range(SC):
                tr = ps_sm.tile([128, 128], BF16, tag="sm")
                nc.tensor.transpose(tr, mix_tok[:, sc, :], ident)
                nc.scalar.copy(out=mixedT[:, sc * 128:(sc + 1) * 128], in_=tr)

            # ---------- w = silu(clip(AI_loc @ AO_full.T)); adapt = w.T chain ---
            ad_ps = ps_poacc.tile([128, T], F32, tag="po")
            wts_buf = {}

            def bmm1_step(j):
                w_ps = ps_big.tile([128, T], F32, tag="mm")
                nc.tensor.matmul(
                    w_ps, AOTf[:, j * 128:(j + 1) * 128], AIT, start=True, stop=True
                )
                wc = work2.tile([128, T], BF16, tag="wc")
                nc.vector.tensor_scalar(
                    out=wc, in0=w_ps, scalar1=-5.0, scalar2=5.0,
                    op0=mybir.AluOpType.max, op1=mybir.AluOpType.min,
                )
                wt = wtp.tile([128, T], BF16, tag="wts")
                nc.scalar.activation(
                    out=wt, in_=wc, func=mybir.ActivationFunctionType.Silu
                )
                wts_buf[j] = wt

            # depth-3 software pipeline: bmm2(j) trails bmm1(j+3) so the PE
            # never waits on the clip/silu stages
            DEPTH = 3
            for j in range(TC_FULL):
                bmm1_step(j)
                if j >= DEPTH:
                    nc.tensor.matmul(
                        ad_ps, AIfull[:, j - DEPTH, :], wts_buf.pop(j - DEPTH),
                        start=(j - DEPTH == 0), stop=False,
                    )
            for j in range(TC_FULL - DEPTH, TC_FULL):
                nc.tensor.matmul(
                    ad_ps, AIfull[:, j, :], wts_buf.pop(j),
                    start=False, stop=(j == TC_FULL - 1),
                )
            nc.scalar.copy(out=adaptT, in_=ad_ps)

            # ---------- finish output ----------
            def final_close(dc):
                op = ps_big.tile([128, T], F32, tag="mm")
                nc.tensor.matmul(
                    op, wfin[:, HC + 1, dc * 128:(dc + 1) * 128], mixedT,
                    start=True, stop=False,
                )
                nc.tensor.matmul(
                    op, wfin[:, HC, dc * 128:(dc + 1) * 128], adaptT,
                    start=False, stop=True,
                )
                ob = evac.tile([128, T], F32, tag="ob")
                nc.vector.tensor_tensor(
                    out=ob, in0=facc[:, dc, :], in1=op, op=mybir.AluOpType.add
                )
                nc.sync.dma_start(out=out_d[dc * 128:(dc + 1) * 128, :], in_=ob)

            for dc in range(DC):
                final_close(dc)

    nc.compile()
    return nc


def kernel(
    x, expert_weights, up_w, gate_w, down_w, pre_w, post_w, an_g, an_b,
    adapt_proj_w, adapter_w, adapter_g, adapter_b, expert_proj_w, output_proj_w,
):
    x = np.asarray(x, np.float32)
    expert_weights = np.asarray(expert_weights, np.float32)
    bf = ml_dtypes.bfloat16

    if "nc" not in _CACHE:
        _CACHE["nc"] = _build()
    nc = _CACHE["nc"]

    def pack(w, kc):
        # [kc*128, F] -> [128, kc, F] (partition-major SBUF layout)
        f = w.shape[1]
        return np.ascontiguousarray(
            w.reshape(kc, 128, f).transpose(1, 0, 2)
        ).astype(bf)

    ug_wT = np.concatenate(
        [np.asarray(up_w, np.float32), np.asarray(gate_w, np.float32)], axis=0
    ).T                                                        # [D, 2H]
    # hc-block-major: [128, 2*HC blocks, KD, 128]
    ug_pack = np.ascontiguousarray(
        ug_wT.reshape(KD, 128, 2 * HC, 128).transpose(1, 2, 0, 3)
    ).astype(bf)
    pre_wT = np.asarray(pre_w, np.float32).T                   # [D, A]
    post_pack = np.ascontiguousarray(
        np.asarray(post_w, np.float32).T.reshape(HC, 128, A).transpose(1, 2, 0)
    ).astype(bf)                                               # [128, A, HC]
    adapter_wT = (
        np.asarray(adapter_w, np.float32).transpose(2, 0, 1).reshape(A, E * A)
    ).astype(bf)                                               # [A, E*A] (e-major)
    down_w = np.asarray(down_w, np.float32)
    w_da = 0.1 * (down_w @ np.asarray(adapt_proj_w, np.float32))       # [D, A]
    w_mo = np.asarray(output_proj_w, np.float32) @ np.asarray(
        expert_proj_w, np.float32
    )                                                                   # [D, A]
    wfin = np.concatenate([down_w.T, w_da.T, w_mo.T], axis=0)  # [2304, D]
    angb = np.stack(
        [np.asarray(an_g, np.float32), np.asarray(an_b, np.float32)], axis=0
    )                                                                   # [2, A]
    ancol = np.ascontiguousarray(angb.T)                                # [A, 2]
    ag_row = np.asarray(adapter_g, np.float32).reshape(1, A * E).astype(bf)  # e-major
    bias_mix = (expert_weights @ np.asarray(adapter_b, np.float32)).astype(bf)

    xf = x.reshape(N, D)
    shared = {
        "ug_wT": ug_pack, "pre_wT": pack(pre_wT, KD),
        "post_wT": post_pack, "adapter_wT": adapter_wT,
        "wfin": pack(wfin, HC + 2), "angb": angb, "ancol": ancol,
        "ag_row": ag_row, "ident": np.eye(128, dtype=bf),
    }
    in_maps = []
    for c in range(NCORES):
        sl = slice(c * T, (c + 1) * T)
        ewc = np.ascontiguousarray(expert_weights[sl]).reshape(SC, 128, E)
        bmc = np.ascontiguousarray(bias_mix[sl]).reshape(SC, 128, A)
        in_maps.append(
            dict(
                shared,
                xT=pack(np.ascontiguousarray(xf[sl].T), KD),
                ew=np.ascontiguousarray(ewc.transpose(1, 0, 2)),
                bias_mix=np.ascontiguousarray(bmc.transpose(1, 0, 2)),
            )
        )

    try:
        res = run_bass_kernel_spmd(nc, in_maps, list(range(NCORES))).results
    except Exception:
        # axon workers occasionally hang up; one retry on a fresh dispatch
        import time

        time.sleep(10)
        res = run_bass_kernel_spmd(nc, in_maps, list(range(NCORES))).results
    out = np.empty((N, D), np.float32)
    for c in range(NCORES):
        out[c * T:(c + 1) * T] = res[c]["out"].T
    return out.reshape(B, S, D)
